# revision 4
# baseline (speedup 1.0000x reference)
"""VMamba SS2D selective-scan kernel for Trainium2 (8 NeuronCores).

Sharding: core c = b*4 + k handles batch b, scan-direction k.
Each core runs the full per-(b,k) pipeline:
  x_dbl = Wproj @ xs ; delta = softplus(dtW @ x_dbl[:R] + bias)
  a = exp(A * delta) ; x_in = (delta*u) * B ; h = scan(a, x_in)
  y = sum_n h*C + Ds*xs
Host does cross-scan layout prep, cross-merge and layernorm (v1).
"""
import numpy as np
import concourse.bass as bass
from concourse import bacc
import concourse.tile as tile
import concourse.mybir as mybir
from concourse.bass_utils import run_bass_kernel_spmd

B, D, H, W = 2, 96, 96, 96
L = H * W
K, N, R = 4, 16, 6
C = R + 2 * N          # 38
NCORES = 8
LC = 1536              # L-chunk size
NCH = L // LC          # 6 chunks
TPD = 8                # d-rows per (d,n) tile
NT = D // TPD          # 12 tiles of 128 partitions
SUB = 512              # PSUM matmul sub-chunk
NSUB = LC // SUB       # 3
EPS = 1e-5

F32 = mybir.dt.float32


def _build():
    nc = bacc.Bacc("TRN2", target_bir_lowering=False, debug=False,
                   num_devices=NCORES)

    xs_d = nc.dram_tensor("xs", [D, L], F32, kind="ExternalInput")
    wproj_d = nc.dram_tensor("wproj", [D, C], F32, kind="ExternalInput")    # lhsT
    dtw_d = nc.dram_tensor("dtw", [R, D], F32, kind="ExternalInput")        # lhsT
    dtb_d = nc.dram_tensor("dtb", [D, 1], F32, kind="ExternalInput")
    alog_d = nc.dram_tensor("alog", [128, NT], F32, kind="ExternalInput")   # A_logs per (tile,p)
    ds_d = nc.dram_tensor("ds", [D, 1], F32, kind="ExternalInput")
    bsel_d = nc.dram_tensor("bsel", [D, NT, 128], F32, kind="ExternalInput")  # broadcast sel (lhsT)
    rsel_d = nc.dram_tensor("rsel", [128, NT, D], F32, kind="ExternalInput")  # reduce sel (lhsT)

    y_d = nc.dram_tensor("y", [D, L], F32, kind="ExternalOutput")

    with tile.TileContext(nc) as tc:
        with (
            tc.tile_pool(name="consts", bufs=1) as consts,
            tc.tile_pool(name="stream", bufs=2) as stream,
            tc.tile_pool(name="bc", bufs=2) as bc,
            tc.tile_pool(name="work", bufs=3) as work,
            tc.tile_pool(name="psA", bufs=2, space="PSUM") as psA,
            tc.tile_pool(name="psB", bufs=2, space="PSUM") as psB,
            tc.tile_pool(name="psY", bufs=1, space="PSUM") as psY,
        ):
            # ---- constants ----
            wproj = consts.tile([D, C], F32)
            nc.sync.dma_start(wproj[:], wproj_d[:])
            dtw = consts.tile([R, D], F32)
            nc.sync.dma_start(dtw[:], dtw_d[:])
            dtb = consts.tile([D, 1], F32)
            nc.sync.dma_start(dtb[:], dtb_d[:])
            ds_c = consts.tile([D, 1], F32)
            nc.sync.dma_start(ds_c[:], ds_d[:])
            bsel = consts.tile([D, NT, 128], F32)
            nc.sync.dma_start(bsel[:], bsel_d[:])
            rsel = consts.tile([128, NT, D], F32)
            nc.sync.dma_start(rsel[:], rsel_d[:])

            alog = consts.tile([128, NT], F32)
            nc.sync.dma_start(alog[:], alog_d[:])
            acol = consts.tile([128, NT], F32)
            # A = -exp(A_logs)
            nc.scalar.activation(acol[:], alog[:],
                                 mybir.ActivationFunctionType.Exp)
            nc.vector.tensor_scalar_mul(acol[:], acol[:], -1.0)

            carry = consts.tile([128, NT], F32)
            nc.vector.memset(carry[:], 0.0)

            for j in range(NCH):
                lo = j * LC
                # ---- load xs chunk ----
                xs = stream.tile([D, LC], F32)
                nc.sync.dma_start(xs[:], xs_d[:, lo:lo + LC])

                # ---- x_dbl = wproj.T @ xs ----
                xdbl = stream.tile([C, LC], F32)
                for s in range(NSUB):
                    ps = psA.tile([C, SUB], F32, tag="psA")
                    nc.tensor.matmul(ps[:], wproj[:], xs[:, s * SUB:(s + 1) * SUB])
                    nc.scalar.copy(xdbl[:, s * SUB:(s + 1) * SUB], ps[:])

                # ---- delta = softplus(dtw.T @ dts_rows + bias) ----
                delta = stream.tile([D, LC], F32)
                for s in range(NSUB):
                    sl = slice(s * SUB, (s + 1) * SUB)
                    ps = psA.tile([D, SUB], F32, tag="psA")
                    nc.tensor.matmul(ps[:], dtw[:], xdbl[0:R, sl])
                    # softplus(x) = ln(1 + exp(x)); x = dts + bias is small
                    spx = work.tile([D, SUB], F32, tag="spx")
                    nc.scalar.activation(spx[:], ps[:],
                                         mybir.ActivationFunctionType.Exp,
                                         bias=dtb[:], scale=1.0)
                    nc.scalar.activation(delta[:, sl], spx[:],
                                         mybir.ActivationFunctionType.Ln,
                                         bias=1.0, scale=1.0)

                # ---- du = delta * xs ----
                du = stream.tile([D, LC], F32)
                nc.vector.tensor_mul(du[:], delta[:], xs[:])

                # ---- b_b, c_b broadcasts (tile-repeat 16-row blocks 8x) ----
                b_b = bc.tile([128, LC], F32)
                c_b = bc.tile([128, LC], F32)
                for r in range(TPD):
                    nc.sync.dma_start(b_b[r * N:(r + 1) * N, :], xdbl[R:R + N, :])
                    nc.sync.dma_start(c_b[r * N:(r + 1) * N, :], xdbl[R + N:C, :])

                # ---- y accumulation target ----
                y_sb = stream.tile([D, LC], F32)
                y_ps = psY.tile([D, LC], F32, tag="psY")

                for t in range(NT):
                    r0 = t * TPD
                    # broadcast delta rows -> [128, SUB] psum; a = exp(A*delta)
                    a_t = work.tile([128, LC], F32, tag="a")
                    xin = work.tile([128, LC], F32, tag="xin")
                    for s in range(NSUB):
                        sl = slice(s * SUB, (s + 1) * SUB)
                        dps = psA.tile([128, SUB], F32, tag="psA")
                        nc.tensor.matmul(dps[:], bsel[:, t, :], delta[:, sl])
                        nc.scalar.activation(a_t[:, sl], dps[:],
                                             mybir.ActivationFunctionType.Exp,
                                             bias=0.0, scale=acol[:, t:t + 1])
                        ups = psB.tile([128, SUB], F32, tag="psB")
                        nc.tensor.matmul(ups[:], bsel[:, t, :], du[:, sl])
                        nc.vector.tensor_mul(xin[:, sl], ups[:], b_b[:, sl])

                    # ---- scan ----
                    h_t = work.tile([128, LC], F32, tag="h")
                    nc.vector.tensor_tensor_scan(
                        h_t[:], a_t[:], xin[:], carry[:, t:t + 1],
                        mybir.AluOpType.mult, mybir.AluOpType.add)
                    nc.vector.tensor_copy(carry[:, t:t + 1], h_t[:, LC - 1:LC])

                    # ---- yprod = h * c_b (gpsimd) ; reduce over n via PE ----
                    yp = work.tile([128, LC], F32, tag="yp")
                    nc.gpsimd.tensor_mul(yp[:], h_t[:], c_b[:])
                    for s in range(NSUB):
                        sl = slice(s * SUB, (s + 1) * SUB)
                        nc.tensor.matmul(y_ps[:, sl], rsel[:, t, :], yp[:, sl],
                                         start=(t == 0), stop=(t == NT - 1))

                for s in range(NSUB):
                    sl = slice(s * SUB, (s + 1) * SUB)
                    nc.scalar.copy(y_sb[:, sl], y_ps[:, sl])

                # ---- skip: y += Ds * xs ----
                nc.vector.scalar_tensor_tensor(
                    y_sb[:], xs[:], ds_c[:], y_sb[:],
                    mybir.AluOpType.mult, mybir.AluOpType.add)

                nc.sync.dma_start(y_d[:, lo:lo + LC], y_sb[:])

    nc.compile()
    return nc


_NC_CACHE = None


def _get_nc():
    global _NC_CACHE
    if _NC_CACHE is None:
        _NC_CACHE = _build()
    return _NC_CACHE


def _make_core_inputs(x, x_proj_weight, dt_projs_weight, dt_projs_bias,
                      A_logs, Ds):
    """Build per-core input dicts (host-side sharding: layout only)."""
    A = A_logs.reshape(K, D, N)
    bsel = np.zeros((D, NT, 128), np.float32)
    for t in range(NT):
        for p in range(128):
            bsel[t * TPD + p // N, t, p] = 1.0
    rsel = np.zeros((128, NT, D), np.float32)
    for t in range(NT):
        for p in range(128):
            rsel[p, t, t * TPD + p // N] = 1.0

    in_maps = []
    for c in range(NCORES):
        b, k = divmod(c, K)
        xb = x[b].reshape(D, L)
        if k == 0:
            xs = xb
        elif k == 1:
            xs = x[b].transpose(0, 2, 1).reshape(D, L)
        elif k == 2:
            xs = xb[:, ::-1]
        else:
            xs = x[b].transpose(0, 2, 1).reshape(D, L)[:, ::-1]
        alog = np.empty((128, NT), np.float32)
        for t in range(NT):
            for p in range(128):
                alog[p, t] = A[k, t * TPD + p // N, p % N]
        in_maps.append(dict(
            xs=np.ascontiguousarray(xs, np.float32),
            wproj=np.ascontiguousarray(x_proj_weight[k].T, np.float32),
            dtw=np.ascontiguousarray(dt_projs_weight[k].T, np.float32),
            dtb=np.ascontiguousarray(dt_projs_bias[k].reshape(D, 1), np.float32),
            alog=alog,
            ds=np.ascontiguousarray(Ds.reshape(K, D)[k].reshape(D, 1), np.float32),
            bsel=bsel,
            rsel=rsel,
        ))
    return in_maps


def _run(inputs, trace=False):
    nc = _get_nc()
    in_maps = _make_core_inputs(
        inputs["x"], inputs["x_proj_weight"], inputs["dt_projs_weight"],
        inputs["dt_projs_bias"], inputs["A_logs"], inputs["Ds"])
    res = run_bass_kernel_spmd(nc, in_maps, core_ids=list(range(NCORES)),
                               trace=trace)
    ys = [res.results[c]["y"] for c in range(NCORES)]

    # host-side unshard: undo orientation, cross-merge, layernorm
    x = inputs["x"]
    out = np.empty((B, L, D), np.float32)
    for b in range(B):
        y0 = ys[b * K + 0]
        y1 = ys[b * K + 1]
        y2 = ys[b * K + 2][:, ::-1]
        y3 = ys[b * K + 3][:, ::-1]
        yhw = y0 + y2
        ywh = y1 + y3
        ywh_t = ywh.reshape(D, W, H).transpose(0, 2, 1).reshape(D, L)
        y = (yhw + ywh_t).T  # [L, D]
        mu = y.mean(-1, keepdims=True)
        var = y.var(-1, keepdims=True)
        y = (y - mu) / np.sqrt(var + EPS)
        y = y * inputs["norm_weight"] + inputs["norm_bias"]
        out[b] = y
    return out.reshape(B, H, W, D), res


def kernel(**inputs):
    out, _ = _run(inputs, trace=False)
    return out
